# revision 1
# baseline (speedup 1.0000x reference)
"""Trainium2 Bass kernel: grouped-pointwise FFN with channel shuffle.

Computes (per batch b, all ops pointwise in T):
    h   = W1_grouped @ (x * mask) + b1          # G=4 block-diagonal GEMM
    h   = channel_shuffle(h, G)
    h   = gelu(h)                               # exact erf gelu
    out = (W2_grouped @ h + b2) * mask

Sharding: data-parallel over batch B=16 across 8 cores (2 batches/core).
Weights are replicated; no collectives.

Layout on device (channel-partition):
  GEMM1: lhsT = w1 block [K=128(cin/G), M=128(out-ch block)],
         rhs  = x tile [128, 512(T chunk)], PSUM out [128, 512].
  gelu+bias fused on ScalarE reading PSUM [128, 1024] spans (2 banks).
  Channel shuffle is free: GEMM2's weight blocks are pre-gathered on the
  host so that GEMM2 group g2 contracts directly over GEMM1's (g, m=g2)
  output tiles.
  GEMM2: accumulate 4 K-blocks into PSUM [128, 512]; drain with a single
  fused DVE op: out = (psum + b2) * mask.

Matmuls stream float32r (fp32 at 1 cycle/row vs 4 for float32; measured
end-to-end rel err ~2e-4 vs fp32 reference). All tensors feeding matmuls
are typed float32r end-to-end (BIR verifier requirement).

DMA: inputs/weights on the SP HWDGE ring (small tensors first, weights
chunked in use-order), outputs on the otherwise-idle GpSimd SWDGE ring.
A burst of tiny warm-up matmuls keeps the PE HAM clock-gate warm before
the first real GEMM.
"""

import numpy as np

import concourse.mybir as mybir
import concourse.tile as tile
from concourse import bacc
from concourse import bass_utils

F32 = mybir.dt.float32
F32R = mybir.dt.float32r

N_CORES = 8
B, CIN, T = 16, 512, 2048
H, COUT, G = 2048, 512, 4
BPC = B // N_CORES        # batches per core
CH = 512                  # T chunk (= max fp32 matmul free dim = 1 PSUM bank)
NCH = T // CH             # 4 chunks
MB = (H // G) // 128      # 4 output-channel blocks per group in GEMM1
GELU_W = 1024             # ACT op width (2 PSUM banks)
XCH = 512                 # x / out DMA chunk width
N_WARMUP = 12             # tiny matmuls to warm the PE clock gate

MM_DT = F32R

_compiled = {}


def _build(mm_dt):
    nc = bacc.Bacc(
        "TRN2", target_bir_lowering=False, debug=False, num_devices=N_CORES
    )
    xs = nc.dram_tensor("xs", [BPC * G, 128, T], mm_dt, kind="ExternalInput").ap()
    mkr = nc.dram_tensor("mkr", [BPC, T], mm_dt, kind="ExternalInput").ap()
    ones = nc.dram_tensor("ones", [1, 128], mm_dt, kind="ExternalInput").ap()
    # w1t columns are (m, g, o)-major so the m=0 block is one contiguous
    # 512-col DMA needed first; w2t columns are (g2, g, o)-major.
    w1t = nc.dram_tensor("w1t", [128, G * MB * 128], mm_dt, kind="ExternalInput").ap()
    w2t = nc.dram_tensor("w2t", [128, G * G * 128], mm_dt, kind="ExternalInput").ap()
    b1t = nc.dram_tensor("b1t", [128, G * MB], F32, kind="ExternalInput").ap()
    b2t = nc.dram_tensor("b2t", [128, G], F32, kind="ExternalInput").ap()
    outs = nc.dram_tensor("outs", [BPC * G, 128, T], F32, kind="ExternalOutput").ap()

    with tile.TileContext(nc) as tc:
        with (
            tc.tile_pool(name="consts", bufs=1) as cpool,
            tc.tile_pool(name="xp", bufs=BPC * G) as xpool,
            tc.tile_pool(name="mbcp", bufs=2) as mbpool,
            tc.tile_pool(name="mkrp", bufs=2) as mkrpool,
            tc.tile_pool(name="hp", bufs=2 * G) as hpool,
            tc.tile_pool(name="op", bufs=2) as opool,
            tc.tile_pool(name="ps1p", bufs=3, space="PSUM") as ps1pool,
            tc.tile_pool(name="ps2p", bufs=2, space="PSUM") as ps2pool,
        ):
            # ones first (warm-up + mask broadcast depend on it)
            ones_sb = cpool.tile([1, 128], mm_dt)
            nc.sync.dma_start(ones_sb, ones)

            # PE warm-up: tiny matmuls on the ones row keep the HAM
            # activity window busy while real inputs stream in.
            wps = ps2pool.tile([128, 128], F32, tag="ps2", name="wps")
            for i in range(N_WARMUP):
                nc.tensor.matmul(
                    wps[:, 0:128], ones_sb, ones_sb, start=True, stop=True
                )

            w1_sb = cpool.tile([128, G * MB * 128], mm_dt)
            w2_sb = cpool.tile([128, G * G * 128], mm_dt)

            x_sb = [[None] * G for _ in range(BPC)]
            mask_bc = [None] * BPC

            def prep_batch(b):
                # mask row -> broadcast across 128 partitions via K=1
                # f32r matmuls, chunk by chunk; x loads and mask muls are
                # chunked so the first GEMM1 matmul can start early.
                mkrow = mkrpool.tile([1, T], mm_dt, tag="mkr", name="mkrow")
                nc.sync.dma_start(mkrow, mkr[b : b + 1, :])
                mbc = mbpool.tile([128, T], F32, tag="mbc", name="mbc")
                for c in range(NCH):
                    cs = slice(c * CH, (c + 1) * CH)
                    psb = ps2pool.tile([128, CH], F32, tag="ps2", name="psb")
                    nc.tensor.matmul(
                        psb, ones_sb, mkrow[:, cs], start=True, stop=True
                    )
                    nc.vector.tensor_copy(mbc[:, cs], psb)
                mask_bc[b] = mbc

            def load_x(b, g, split_first=False, ring=None):
                # ring: engine issuing the DMAs; later groups go out on
                # the otherwise-idle GpSimd SWDGE ring so both rings
                # issue in parallel during the head
                ring = nc.sync if ring is None else ring
                xt = xpool.tile([128, T], mm_dt, tag="x", name="xt")
                start_c = 0
                if split_first:
                    # first chunk as two 128KB DMAs so they land on
                    # parallel queues and GEMM1 can start sooner
                    for h in range(2):
                        hs = slice(h * (XCH // 2), (h + 1) * (XCH // 2))
                        ring.dma_start(xt[:, hs], xs[b * G + g][:, hs])
                        nc.vector.tensor_mul(
                            xt[:, hs], xt[:, hs], mask_bc[b][:, hs]
                        )
                    start_c = 1
                for c in range(start_c, T // XCH):
                    cs = slice(c * XCH, (c + 1) * XCH)
                    ring.dma_start(xt[:, cs], xs[b * G + g][:, cs])
                    nc.vector.tensor_mul(
                        xt[:, cs], xt[:, cs], mask_bc[b][:, cs]
                    )
                x_sb[b][g] = xt

            def load_w1(m):
                ws = slice(m * G * 128, (m + 1) * G * 128)
                nc.sync.dma_start(w1_sb[:, ws], w1t[:, ws])

            def load_w2(g2):
                ws = slice(g2 * G * 128, (g2 + 1) * G * 128)
                nc.sync.dma_start(w2_sb[:, ws], w2t[:, ws])

            def gemm1_g(b, m, g):
                # one h tile (g) for (b, m), gelu+bias fused on drain
                ht = hpool.tile([128, T], mm_dt, tag="h", name="ht")
                w_ap = w1_sb[:, (m * G + g) * 128 : (m * G + g + 1) * 128]
                for half in range(T // GELU_W):
                    ps1 = ps1pool.tile([128, GELU_W], F32, tag="ps1", name="ps1")
                    for cc in range(GELU_W // CH):
                        c = half * (GELU_W // CH) + cc
                        nc.tensor.matmul(
                            ps1[:, cc * CH : (cc + 1) * CH],
                            w_ap,
                            x_sb[b][g][:, c * CH : (c + 1) * CH],
                            start=True, stop=True,
                        )
                    nc.scalar.activation(
                        ht[:, half * GELU_W : (half + 1) * GELU_W],
                        ps1,
                        mybir.ActivationFunctionType.Gelu,
                        bias=b1_sb[:, m * G + g : m * G + g + 1],
                        scale=1.0,
                    )
                return ht

            def gemm2_chunk(b, g2, hts, ot, c, och=XCH * 2):
                cs = slice(c * CH, (c + 1) * CH)
                ps2 = ps2pool.tile([128, CH], F32, tag="ps2", name="ps2")
                for g in range(G):
                    nc.tensor.matmul(
                        ps2,
                        w2_sb[:, (g2 * G + g) * 128 : (g2 * G + g + 1) * 128],
                        hts[g][:, cs],
                        start=(g == 0), stop=(g == G - 1),
                    )
                # out = (psum + b2) * mask, single fused DVE op
                nc.vector.scalar_tensor_tensor(
                    ot[:, cs],
                    ps2,
                    b2_sb[:, g2 : g2 + 1],
                    mask_bc[b][:, cs],
                    op0=mybir.AluOpType.add,
                    op1=mybir.AluOpType.mult,
                )
                if ((c + 1) * CH) % och == 0:
                    os_ = slice((c + 1) * CH - och, (c + 1) * CH)
                    nc.sync.dma_start(outs[b * G + g2][:, os_], ot[:, os_])

            # head: mask prep + first x tiles, weight blocks in use-order;
            # batch 1 is prefetched entirely up front too (its broadcast
            # matmuls double as PE warm-up while DMAs stream)
            prep_batch(0)
            load_x(0, 0, split_first=True)
            load_x(0, 2, split_first=True, ring=nc.gpsimd)
            b1_sb = cpool.tile([128, G * MB], F32)
            nc.sync.dma_start(b1_sb, b1t)
            b2_sb = cpool.tile([128, G], F32)
            nc.sync.dma_start(b2_sb, b2t)
            load_w1(0)
            load_x(0, 1)
            load_x(0, 3, ring=nc.gpsimd)
            load_w1(1)
            load_w1(2)
            load_w1(3)
            for g2 in range(G):
                load_w2(g2)

            # software pipeline over (b, m): GEMM2 chunks of iteration i-1
            # are interleaved between GEMM1 groups of iteration i so PE
            # alternates with ScalarE instead of stalling on gelu.
            prev = None
            for b in range(BPC):
                for m in range(MB):
                    hts = []
                    if prev is not None:
                        pot = opool.tile([128, T], F32, tag="o", name="pot")
                    for g in range(G):
                        hts.append(gemm1_g(b, m, g))
                        if prev is not None:
                            gemm2_chunk(prev[0], prev[1], prev[2], pot, g)
                    prev = (b, m, hts)
                    if b + 1 < BPC and m == 1:
                        prep_batch(b + 1)
                        for g in range(G):
                            load_x(
                                b + 1, g,
                                ring=nc.gpsimd if g >= 2 else None,
                            )
            pot = opool.tile([128, T], F32, tag="o", name="pot")
            for c in range(NCH):
                gemm2_chunk(prev[0], prev[1], prev[2], pot, c, och=CH)

    nc.compile()
    return nc


def get_nc(mm_dt=None):
    mm_dt = MM_DT if mm_dt is None else mm_dt
    if mm_dt not in _compiled:
        _compiled[mm_dt] = _build(mm_dt)
    return _compiled[mm_dt]


def prep_inputs(x, x_mask, w1, b1, w2, b2):
    """Host-side layout prep. Returns per-core in_maps."""
    x = np.ascontiguousarray(np.asarray(x, dtype=np.float32))
    x_mask = np.asarray(x_mask, dtype=np.float32)
    w1 = np.asarray(w1, dtype=np.float32)
    b1 = np.asarray(b1, dtype=np.float32)
    w2 = np.asarray(w2, dtype=np.float32)
    b2 = np.asarray(b2, dtype=np.float32)

    # w1 [H, CIN/G] -> lhsT blocks [i, (m, g, o)]
    w1r = w1.reshape(G, MB, 128, CIN // G)          # g, m, o, i
    w1t = np.ascontiguousarray(
        np.transpose(w1r, (3, 1, 0, 2)).reshape(128, G * MB * 128)
    )
    # w2 [COUT, H/G] -> lhsT blocks [i_local, (g2, g, o)]
    # GEMM2 group g2 contracts h tile (g, m=g2) row r against
    # w2[g2*128+o, r*4+g] (channel shuffle pre-applied).
    w2r = w2.reshape(G, 128, 128, G)                # g2, o, r, g
    w2t = np.ascontiguousarray(
        np.transpose(w2r, (2, 0, 3, 1)).reshape(128, G * G * 128)
    )
    b1tt = np.ascontiguousarray(
        b1.reshape(G, MB, 128).transpose(2, 1, 0).reshape(128, G * MB)
    )
    b2tt = np.ascontiguousarray(b2.reshape(G, 128).T)
    ones = np.ones((1, 128), np.float32)

    xr = x.reshape(N_CORES, BPC * G, 128, T)
    mr = x_mask.reshape(N_CORES, BPC, T)

    in_maps = []
    for k in range(N_CORES):
        mk_k = np.ascontiguousarray(mr[k])
        in_maps.append(
            {
                "xs": np.ascontiguousarray(xr[k]),
                "mkr": mk_k,
                "ones": ones,
                "w1t": w1t,
                "w2t": w2t,
                "b1t": b1tt,
                "b2t": b2tt,
            }
        )
    return in_maps


def assemble_output(results):
    """results: list of 8 dicts with 'outs' [BPC*G, 128, T]."""
    parts = [r["outs"].reshape(BPC, G * 128, T) for r in results]
    return np.concatenate(parts, axis=0).astype(np.float32)


def kernel(x, x_mask, w1, b1, w2, b2, n_groups):
    assert int(n_groups) == G
    import os

    # NTFF tracing needs antenv.axon_hooks, absent on this image; make
    # sure an inherited BASS_TRACE can't push us onto that path.
    os.environ["BASS_NEVER_TRACE"] = "1"
    nc = get_nc()
    in_maps = prep_inputs(x, x_mask, w1, b1, w2, b2)
    res = bass_utils.run_bass_kernel_spmd(
        nc, in_maps, core_ids=list(range(N_CORES))
    )
    return assemble_output(res.results)



# revision 3
# speedup vs baseline: 1.2924x; 1.2924x over previous
"""Trainium2 Bass kernel: grouped-pointwise FFN with channel shuffle.

Computes (per batch b, all ops pointwise in T):
    h   = W1_grouped @ (x * mask) + b1          # G=4 block-diagonal GEMM
    h   = channel_shuffle(h, G)
    h   = gelu(h)                               # exact erf gelu
    out = (W2_grouped @ h + b2) * mask

Sharding: data-parallel over batch B=16 across 8 cores (2 batches/core).
Weights are replicated; no collectives.

The spec pins x_mask to all-ones, so the kernel drops the mask path on
device; if a caller ever passes a non-trivial mask it is applied exactly
on the host (x*mask pre, out*mask post) which commutes with the kernel.

Engine budget per core (pace analysis):
  ACT (gelu, 1.2GHz, dtype-independent): 64 ops x [128,1024] ~ 68.5us  <- pace
  PE  (256 matmuls x 512 free, bf16 1cyc/row @2.4GHz): ~54.6us
  DVE (GEMM2 drain +bias): 32 x [128,512] ~ 22.5us
  DMA (bf16 in 5.2MB, f32 out 8.4MB): ~41us wire
ACT is the critical engine; the schedule keeps it streaming back-to-back:
GEMM1 halves feed a 3-buf [128,1024] PSUM pool (6 banks), GEMM2 uses a
2-buf [128,512] pool (2 banks), GEMM2 of iteration i interleaves with
GEMM1 of iteration i+1 (lag 1) except the last iteration which
interleaves within itself to shorten the tail.

Channel shuffle is free: GEMM2's weight blocks are pre-gathered on the
host so GEMM2 group g2 contracts directly over GEMM1's (g, m=g2) tiles.

All matmul operands are bf16 (same PE rate as fp32r; half the DMA bytes
and SBUF); PSUM stays fp32, gelu output h is bf16, final out fp32.
"""

import numpy as np

import concourse.mybir as mybir
import concourse.tile as tile
from concourse import bacc
from concourse import bass_utils

F32 = mybir.dt.float32
BF16 = mybir.dt.bfloat16

N_CORES = 8
B, CIN, T = 16, 512, 2048
H, COUT, G = 2048, 512, 4
BPC = B // N_CORES        # batches per core
MB = (H // G) // 128      # 4 output-channel blocks per group in GEMM1
CH = 512                  # matmul free dim (1 PSUM bank)
AW = 1024                 # ACT op width (2 PSUM banks)

MM_DT = BF16

_compiled = {}


def _build(mm_dt):
    nc = bacc.Bacc(
        "TRN2", target_bir_lowering=False, debug=False, num_devices=N_CORES
    )
    xs = nc.dram_tensor("xs", [BPC * G, 128, T], mm_dt, kind="ExternalInput").ap()
    # wpk cols: w1t [(m, g, o)-major, 2048] then w2t [(g2, g, o)-major, 2048]
    wpk = nc.dram_tensor("wpk", [128, 2 * G * MB * 128], mm_dt, kind="ExternalInput").ap()
    # bpk cols: b1t [m*G+g, 16] then b2t [g2, 4]
    bpk = nc.dram_tensor("bpk", [128, G * MB + G], F32, kind="ExternalInput").ap()
    outs = nc.dram_tensor("outs", [BPC * G, 128, T], F32, kind="ExternalOutput").ap()

    with tile.TileContext(nc) as tc:
        with (
            tc.tile_pool(name="consts", bufs=1) as cpool,
            tc.tile_pool(name="xp", bufs=BPC * G) as xpool,
            tc.tile_pool(name="hp", bufs=2 * G) as hpool,
            tc.tile_pool(name="op", bufs=2) as opool,
            tc.tile_pool(name="ps1p", bufs=3, space="PSUM") as ps1pool,
            tc.tile_pool(name="ps2p", bufs=2, space="PSUM") as ps2pool,
        ):
            w_sb = cpool.tile([128, 2 * G * MB * 128], mm_dt)
            b_sb = cpool.tile([128, G * MB + G], F32)
            x_sb = [[None] * G for _ in range(BPC)]

            # head DMAs: the two first-need chunks go out in parallel on
            # separate rings (w1 m=0 on sync, x(0,0) on gpsimd), then the
            # rest in use-order.
            nc.sync.dma_start(w_sb[:, 0 : G * 128], wpk[:, 0 : G * 128])
            xt00 = xpool.tile([128, T], mm_dt, tag="x", name="xt")
            nc.gpsimd.dma_start(xt00[:, 0:CH], xs[0][:, 0:CH])
            nc.sync.dma_start(b_sb, bpk)
            nc.gpsimd.dma_start(xt00[:, CH:T], xs[0][:, CH:T])
            x_sb[0][0] = xt00
            nc.sync.dma_start(
                w_sb[:, G * 128 : G * MB * 128], wpk[:, G * 128 : G * MB * 128]
            )
            for g in range(1, G):
                xt = xpool.tile([128, T], mm_dt, tag="x", name="xt")
                nc.sync.dma_start(xt, xs[g])
                x_sb[0][g] = xt
            nc.sync.dma_start(
                w_sb[:, G * MB * 128 :], wpk[:, G * MB * 128 :]
            )

            def load_x(b, g):
                xt = xpool.tile([128, T], mm_dt, tag="x", name="xt")
                nc.sync.dma_start(xt, xs[b * G + g])
                x_sb[b][g] = xt

            def g1_half(b, m, g, half, ht):
                # one [128,1024] PSUM tile: 2 matmuls + fused gelu/bias
                ps1 = ps1pool.tile([128, AW], F32, tag="ps1", name="ps1")
                w_ap = w_sb[:, (m * G + g) * 128 : (m * G + g + 1) * 128]
                for c2 in range(AW // CH):
                    c = half * (AW // CH) + c2
                    nc.tensor.matmul(
                        ps1[:, c2 * CH : (c2 + 1) * CH],
                        w_ap,
                        x_sb[b][g][:, c * CH : (c + 1) * CH],
                        start=True, stop=True,
                    )
                nc.scalar.activation(
                    ht[:, half * AW : (half + 1) * AW],
                    ps1,
                    mybir.ActivationFunctionType.Gelu,
                    bias=b_sb[:, m * G + g : m * G + g + 1],
                    scale=1.0,
                )

            def g2_chunk(b, g2, hts, ot, c):
                cs = slice(c * CH, (c + 1) * CH)
                ps2 = ps2pool.tile([128, CH], F32, tag="ps2", name="ps2")
                for g in range(G):
                    wo = G * MB * 128 + (g2 * G + g) * 128
                    nc.tensor.matmul(
                        ps2,
                        w_sb[:, wo : wo + 128],
                        hts[g][:, cs],
                        start=(g == 0), stop=(g == G - 1),
                    )
                # out = psum + b2 (per-partition scalar) on DVE
                nc.vector.tensor_scalar_add(
                    ot[:, cs],
                    ps2,
                    b_sb[:, G * MB + g2 : G * MB + g2 + 1],
                )
                if c % 2 == 1:  # store half-tiles
                    os_ = slice((c - 1) * CH, (c + 1) * CH)
                    nc.gpsimd.dma_start(outs[b * G + g2][:, os_], ot[:, os_])

            # pipeline: GEMM2 of iter i-1 interleaves with GEMM1 of iter i;
            # the last iteration interleaves its own GEMM2 (after the h
            # halves it needs) to shorten the tail.
            NIT = BPC * MB
            prev = None
            for it in range(NIT):
                b, m = divmod(it, MB)
                last = it == NIT - 1
                hts = [
                    hpool.tile([128, T], mm_dt, tag="h", name="ht")
                    for _ in range(G)
                ]
                if prev is not None:
                    pot = opool.tile([128, T], F32, tag="o", name="pot")
                if not last:
                    for g in range(G):
                        g1_half(b, m, g, 0, hts[g])
                        g1_half(b, m, g, 1, hts[g])
                        if prev is not None:
                            g2_chunk(prev[0], prev[1], prev[2], pot, g)
                else:
                    # halves-first order so own-GEMM2 can start early
                    for g in range(G):
                        g1_half(b, m, g, 0, hts[g])
                        if g < 2 and prev is not None:
                            g2_chunk(prev[0], prev[1], prev[2], pot, 2 * g)
                            g2_chunk(prev[0], prev[1], prev[2], pot, 2 * g + 1)
                    for g in range(G):
                        g1_half(b, m, g, 1, hts[g])
                # x prefetch for batch b+1 spread over early iterations
                if b + 1 < BPC and m in (1, 2):
                    for g in range(2):
                        load_x(b + 1, 2 * (m - 1) + g)
                prev = (b, m, hts)
            # tail: GEMM2 of the last iteration
            pot = opool.tile([128, T], F32, tag="o", name="pot")
            for c in range(T // CH):
                g2_chunk(prev[0], prev[1], prev[2], pot, c)

    nc.compile()
    return nc


def get_nc(mm_dt=None):
    mm_dt = MM_DT if mm_dt is None else mm_dt
    if mm_dt not in _compiled:
        _compiled[mm_dt] = _build(mm_dt)
    return _compiled[mm_dt]


def prep_inputs(x, x_mask, w1, b1, w2, b2):
    """Host-side layout prep. Returns per-core in_maps."""
    import ml_dtypes

    bf16 = ml_dtypes.bfloat16
    x = np.asarray(x, dtype=np.float32)
    w1 = np.asarray(w1, dtype=np.float32)
    b1 = np.asarray(b1, dtype=np.float32)
    w2 = np.asarray(w2, dtype=np.float32)
    b2 = np.asarray(b2, dtype=np.float32)

    # w1 [H, CIN/G] -> lhsT blocks [i, (m, g, o)]
    w1r = w1.reshape(G, MB, 128, CIN // G)          # g, m, o, i
    w1t = np.transpose(w1r, (3, 1, 0, 2)).reshape(128, G * MB * 128)
    # w2 [COUT, H/G] -> lhsT blocks [r, (g2, g, o)]; GEMM2 group g2
    # contracts h tile (g, m=g2) row r against w2[g2*128+o, r*G+g]
    # (channel shuffle pre-applied).
    w2r = w2.reshape(G, 128, 128, G)                # g2, o, r, g
    w2t = np.transpose(w2r, (2, 0, 3, 1)).reshape(128, G * G * 128)
    wpk = np.ascontiguousarray(
        np.concatenate([w1t, w2t], axis=1).astype(bf16)
    )
    b1t = b1.reshape(G, MB, 128).transpose(2, 1, 0).reshape(128, G * MB)
    b2t = b2.reshape(G, 128).T
    bpk = np.ascontiguousarray(
        np.concatenate([b1t, b2t], axis=1).astype(np.float32)
    )

    xr = np.ascontiguousarray(
        x.reshape(N_CORES, BPC * G, 128, T).astype(bf16)
    )

    in_maps = []
    for k in range(N_CORES):
        in_maps.append({"xs": xr[k], "wpk": wpk, "bpk": bpk})
    return in_maps


def assemble_output(results):
    """results: list of 8 dicts with 'outs' [BPC*G, 128, T]."""
    parts = [r["outs"].reshape(BPC, G * 128, T) for r in results]
    return np.concatenate(parts, axis=0).astype(np.float32)


def kernel(x, x_mask, w1, b1, w2, b2, n_groups):
    assert int(n_groups) == G
    import os

    # NTFF tracing needs antenv.axon_hooks, absent on this image; make
    # sure an inherited BASS_TRACE can't push us onto that path.
    os.environ["BASS_NEVER_TRACE"] = "1"

    x = np.asarray(x, dtype=np.float32)
    x_mask = np.asarray(x_mask, dtype=np.float32)
    trivial_mask = bool(np.all(x_mask == 1.0))
    if not trivial_mask:
        # mask is per-(b,t): it commutes with the pointwise convs, so
        # exact host-side pre/post multiply preserves semantics.
        x = x * x_mask

    nc = get_nc()
    in_maps = prep_inputs(x, x_mask, w1, b1, w2, b2)
    res = bass_utils.run_bass_kernel_spmd(
        nc, in_maps, core_ids=list(range(N_CORES))
    )
    out = assemble_output(res.results)
    if not trivial_mask:
        out = out * x_mask
    return out
